# revision 46
# baseline (speedup 1.0000x reference)
"""Multi-head attention (B=2, S=2048, D=1024, H=16) on 8 Trainium2 cores.

Sharding: tensor-parallel over heads (4 groups of 4 heads) x data-parallel
over batch (2). Core c handles batch c//4, head group c%4. Out-projection:
each core computes fp16 partials for all 1024 out cols from its 256 ctx
dims; ReduceScatter(add) over the 4-core group hands rank r its 256-col
quarter.

v2 pipeline (all f16 activations/weights, fp32 PSUM):
  per sk/sq group j: project k(j), v(j), q(j) from x pieces, then attention
  jg=j. Proj/out-proj matmul chains are emitted as *fillers* between
  attention blocks so TensorE never idles while ScalarE runs exp (keeps the
  HAM clock-gate warm). Scores for a head-pair land in one 2-bank PSUM tile
  [128,2,512] so a single ACT exp covers both heads. Causal masking is done
  after exp by GpSimd affine_select (zeroes the upper triangle of the diag
  block) -- no mask tensor, no VectorE work. The V tiles carry a ones
  column so the PV matmul accumulates softmax denominators for free; the
  reciprocal uses the fast DVE approx (1 op) and is broadcast across
  partitions with a tiny ones-vector matmul.
"""
import os
from collections import deque

import numpy as np

import concourse.bass as bass
import concourse.mybir as mybir
import concourse.tile as tile
import bass_rust as _bass_rust
from concourse.bass_utils import run_bass_kernel_spmd

dt = mybir.dt
AF = mybir.ActivationFunctionType
ALU = mybir.AluOpType

B, S, D, H = 2, 2048, 1024, 16
DK = D // H          # 64
HL = 4               # heads per core
DL = HL * DK         # 256 local head dims
NCORE = 8
GROUPS = [[0, 1, 2, 3], [4, 5, 6, 7]]
SQG = 512            # sq group width (one PSUM bank of fp32)
NSQG = S // SQG      # 4
NSK = S // 128       # 16 sk blocks
KCH = D // 128       # 8 contraction chunks for projections
# NOTE: fp8 on x/w was tried (XS=16, WS=32, DoubleRow) and FAILS accuracy:
# attention ctx is an average, so signal shrinks ~1/sqrt(n_eff) exactly like
# the quantization noise -- per-element fp8 error (~4%) lands full-strength
# on the output (measured rel err 6.3e-2 vs the 2e-2 gate). Keep f16.
XS, WS = 1.0, 1.0
PSC = XS * WS
SCALE = 1.0 / float(np.sqrt(np.float32(DK))) / (PSC * PSC)

F16 = dt.float16
F32 = dt.float32
F32R = dt.float32r
F8 = dt.float8e4

LAST_RESULT = None   # BassKernelResults of the most recent run (profiling)
_CACHE = {}          # causal -> built Bass


def _split_multiwait(nc):
    """This walrus supports one sync-wait per instruction; Tile emits several.
    Hoist all but the last wait of each instruction onto single-wait NOPs
    placed immediately before it on the same engine."""
    for bbw in nc.bb_map.values():
        insts = bbw.bb.instructions
        out = []
        for inst in insts:
            si = inst.sync_info
            waits = list(si.on_wait or []) if si is not None else []
            if len(waits) > 1:
                for w in waits[:-1]:
                    nop = _bass_rust.InstNoOp(
                        name=nc.get_next_instruction_name(), ins=[], outs=[])
                    nop.engine = inst.engine
                    nop.bass_nofuse = True
                    nop.sync_info = mybir.SyncInfo(on_wait=[w], on_update=[])
                    nc.register_instruction(nop)
                    out.append(nop)
                inst.sync_info = mybir.SyncInfo(
                    on_wait=[waits[-1]], on_update=list(si.on_update or []))
            out.append(inst)
        insts[:] = out


def _build(causal: bool):
    nc = bass.Bass(num_devices=NCORE)

    # x tensors arrive host-packed as [128, NSQG, KCH, SQG] and weights as
    # [128, KCH, DL] so every staging DMA moves contiguous 4-8KB runs per
    # partition at full HBM line rate (512B-row rearranges measured ~60GB/s).
    xq = nc.declare_dram_parameter("xq", [128, NSQG, KCH, SQG], F16, isOutput=False)
    xk = nc.declare_dram_parameter("xk", [128, NSQG, KCH, SQG], F16, isOutput=False)
    xv = nc.declare_dram_parameter("xv", [128, NSQG, KCH, SQG], F16, isOutput=False)
    wq = nc.declare_dram_parameter("wq", [128, KCH, DL], F16, isOutput=False)
    wk = nc.declare_dram_parameter("wk", [128, KCH, DL], F16, isOutput=False)
    wv = nc.declare_dram_parameter("wv", [128, KCH, DL], F16, isOutput=False)
    wo = nc.declare_dram_parameter("wo", [128, KCH, DL], F16, isOutput=False)
    out = nc.declare_dram_parameter("out", [2, 128, S], F16, isOutput=True)
    debug = os.environ.get("KERNEL_DEBUG", "0") == "1"
    if debug:
        d_qT = nc.declare_dram_parameter("d_qT", [2, 128, S], F16, isOutput=True)
        d_kT = nc.declare_dram_parameter("d_kT", [2, 128, S], F16, isOutput=True)
        d_Vp = nc.declare_dram_parameter("d_Vp", [128, NSK, 65 * HL], F16, isOutput=True)
        d_ctx = nc.declare_dram_parameter("d_ctx", [128, 2, S], F16, isOutput=True)

    with tile.TileContext(nc) as tc:
        with (
            tc.tile_pool(name="wpool", bufs=1) as wpool,
            tc.tile_pool(name="xpool", bufs=4) as xpool,
            tc.tile_pool(name="apool", bufs=1) as apool,
            tc.tile_pool(name="epool", bufs=4) as epool,
            tc.tile_pool(name="opool", bufs=2) as opool,
            tc.tile_pool(name="psS", bufs=2, space="PSUM") as psS,
            tc.tile_pool(name="psC", bufs=1, space="PSUM") as psC,
            tc.tile_pool(name="psP", bufs=1, space="PSUM") as psP,
            tc.tile_pool(name="dram", bufs=1, space="DRAM") as drp,
        ):
            # ---- resident weights / constants ----
            # NOTHING stages on the scalar queue: every DIRECT2D issued there
            # (~0.7us each + ring-full stalls) delays all subsequent EXPs --
            # measured to push the first exp to t=66us with 32 loads queued.
            wq_sb = wpool.tile([128, KCH, DL], F16, tag="wq")
            wk_sb = wpool.tile([128, KCH, DL], F16, tag="wk")
            wv_sb = wpool.tile([128, KCH, DL], F16, tag="wv")
            wo_sb = wpool.tile([128, KCH, DL], F16, tag="wo")
            nc.sync.dma_start(wk_sb[:], wk[:])
            nc.gpsimd.dma_start(wq_sb[:], wq[:])
            ones64f = wpool.tile([1, 64], F32, tag="ones64f")
            nc.vector.memset(ones64f[:], 1.0)
            ones64 = wpool.tile([1, 64], F32R, tag="ones64")
            nc.vector.tensor_copy(ones64[:], ones64f[:])
            # 0/1 strictly-lower-triangular (keep f >= p) mask, doubled along
            # the head axis; built once on gpsimd while its queue is empty
            mask01 = wpool.tile([128, 2, 128], F16, tag="mask01")
            nc.vector.memset(mask01[:], 1.0)
            nc.gpsimd.affine_select(
                mask01[:], mask01[:], pattern=[[0, 2], [1, 128]],
                compare_op=ALU.is_ge, fill=0.0, base=0,
                channel_multiplier=-1)

            # ---- persistent activations ----
            qT = [apool.tile([128, S], F16, tag=f"qT{hp}", name=f"qT{hp}")
                  for hp in range(2)]
            kT = [apool.tile([128, S], F16, tag=f"kT{hp}", name=f"kT{hp}")
                  for hp in range(2)]
            Vp = apool.tile([128, NSK, 65 * HL], F16, tag="Vp")
            nc.gpsimd.memset(
                Vp.rearrange("p i (h e) -> p i h e", e=65)[:, :, :, 64:65], 1.0)
            ctx_sb = apool.tile([128, 2, S], F16, tag="ctx")

            # ---- x resident: [128, NSQG, KCH, SQG] per tensor ----
            # Staged per 512-col sq/sk group, ONE DMA per (tensor, group),
            # fully contiguous on both sides (8KB per partition): HWDGE
            # rings drain FIFO per issuing engine, so group 0 completes
            # first and proj(0)/attn(0) start early while groups 1-3 stream
            # in behind. Split across the sync and gpsimd queues only (the
            # scalar queue stays clear for EXPs).
            xk_sb = xpool.tile([128, NSQG, KCH, SQG], F16, tag="xk", bufs=1)
            xv_sb = xpool.tile([128, NSQG, KCH, SQG], F16, tag="xv", bufs=1)
            xq_sb = xpool.tile([128, NSQG, KCH, SQG], F16, tag="xq", bufs=1)
            for j in range(NSQG):
                for eng, tl, xsrc in ((nc.sync, xk_sb, xk),
                                      (nc.gpsimd, xq_sb, xq),
                                      (nc.sync, xv_sb, xv)):
                    eng.dma_start(tl[:, j], xsrc[:, j])
                if j == 0:
                    # wv after xq0: v inputs aren't needed until attn(0)
                    # block ~3, while q proj gates the very first scores.
                    nc.gpsimd.dma_start(wv_sb[:], wv[:])
            # wo is first needed by outproj(0) during attn(2); stage it last
            nc.gpsimd.dma_start(wo_sb[:], wo[:])
            xk_pc = [xk_sb[:, j] for j in range(NSQG)]
            xv_pc = [xv_sb[:, j] for j in range(NSQG)]
            xq_pc = [xq_sb[:, j] for j in range(NSQG)]

            # ---------------- filler chain machinery ----------------
            # Each chain-unit is a list of thunks; consecutive thunks of the
            # open unit are popped between attention blocks. A unit owns one
            # psP tile for its whole life, so units must not interleave.
            fillers = deque()   # deque of lists (chain units); unit = deque of thunks
            pending_tail = deque()  # finalize tails queued for the NEXT group

            def drain(n):
                """Emit up to n filler thunks (crossing unit boundaries)."""
                while n > 0 and fillers:
                    unit = fillers[0]
                    while n > 0 and unit:
                        unit.popleft()()
                        n -= 1
                    if not unit:
                        fillers.popleft()

            def drain_unit_boundary():
                """Finish the currently open chain unit (frees its psP tile)."""
                if fillers and fillers[0]:
                    unit = fillers.popleft()
                    while unit:
                        unit.popleft()()

            def drain_all():
                while fillers:
                    drain_unit_boundary()

            # ---------------- projection chain units ----------------
            def qk_proj_unit(j, xt, w_sb, dst):
                """One unit: both 128-row halves of q/k columns for group j."""
                unit = deque()
                state = {}

                def open_():
                    state["ps"] = psP.tile([128, 2, SQG], F32, tag="pj",
                                           name=f"pj_{id(state)}")
                for kk in range(KCH):
                    def mm(kk=kk):
                        if kk == 0:
                            open_()
                        ps = state["ps"]
                        for cc in range(2):
                            nc.tensor.matmul(
                                ps[:, cc, :],
                                lhsT=w_sb[:, kk, 128 * cc:128 * (cc + 1)],
                                rhs=xt[:, kk, :],
                                start=(kk == 0), stop=(kk == KCH - 1))
                    unit.append(mm)

                def close():
                    ps = state["ps"]
                    for cc in range(2):
                        nc.vector.tensor_copy(
                            dst[cc][:, SQG * j:SQG * (j + 1)], ps[:, cc, :])
                unit.append(close)
                return unit

            def v_proj_unit(j, half):
                """One unit: two sk-128-chunks of v for group j (natural)."""
                unit = deque()
                state = {}
                for kk in range(KCH):
                    def mm(kk=kk):
                        if kk == 0:
                            state["ps"] = psP.tile([128, 2, SQG], F32, tag="pj",
                                                   name=f"pv_{id(state)}")
                        ps = state["ps"]
                        for sc2 in range(2):
                            sc = 2 * half + sc2
                            nc.tensor.matmul(
                                ps[:, sc2, :DL],
                                lhsT=xv_pc[j][:, kk, 128 * sc:128 * (sc + 1)],
                                rhs=wv_sb[:, kk, :],
                                start=(kk == 0), stop=(kk == KCH - 1))
                    unit.append(mm)

                def close():
                    ps = state["ps"]
                    for sc2 in range(2):
                        sc = 2 * half + sc2
                        i = 4 * j + sc
                        vdst = Vp[:, i].rearrange("p (h e) -> p h e", e=65)
                        nc.vector.tensor_copy(
                            vdst[:, :, :64],
                            ps[:, sc2, :DL].rearrange("p (h e) -> p h e", e=64))
                unit.append(close)
                return unit

            def proj_units(j):
                """Chain units for group j in dependency-useful order."""
                return [
                    qk_proj_unit(j, xk_pc[j], wk_sb, kT),
                    qk_proj_unit(j, xq_pc[j], wq_sb, qT),
                    v_proj_unit(j, 0),
                    v_proj_unit(j, 1),
                ]

            # ------------- out-projection via AllGather of ctx -------------
            # After both finalizes of group jg, the normalized ctx is written
            # to DRAM and AllGathered over the 4-core group (AG moves half
            # the wire bytes of a ReduceScatter and needs no pre-collective
            # matmuls, so it fires earlier and the tail shrinks). ONE 256KB
            # AG per jg (not one per hp): the mesh algo has a ~10us latency
            # floor regardless of size, and the 8 per-hp AGs saturated the
            # serial CC queue. Each core then computes only its own 256 out
            # columns from the gathered 1024 ctx rows.
            ctxg_dr = {}

            def ship_ctx(jg):
                ctxdr = drp.tile([128, 2, SQG], F16, name=f"ctxdr{jg}")
                with tc.high_priority():
                    nc.scalar.dma_start(
                        ctxdr[:], ctx_sb[:, :, SQG * jg:SQG * (jg + 1)])
                ctg = drp.tile([4, 128, 2, SQG], F16, name=f"ctxgd{jg}")
                nc.gpsimd.collective_compute(
                    "AllGather", ALU.bypass, replica_groups=GROUPS,
                    ins=[ctxdr.opt()], outs=[ctg.opt()])
                ctxg_dr[jg] = ctg

            def outproj_units(jg):
                st = {}
                units = []
                uL = deque()

                def loadg():
                    # On the SYNC queue: this DMA blocks in-order on the
                    # AllGather's completion sem, and sync now carries only
                    # AG-ordered consumers (ctxg loads, out ships) after
                    # startup staging -- so the stall never strands an
                    # independent op. On scalar it stalled every subsequent
                    # EXP ~15us/jg; on gpsimd it strands the finalize
                    # partition_broadcasts (and with them the ctx ships).
                    for hp in range(2):
                        g = opool.tile([128, 4, SQG], F16, tag=f"ctxg{hp}",
                                       name=f"ctxg{jg}_{hp}")
                        nc.sync.dma_start(
                            g[:],
                            ctxg_dr[jg][:, :, hp, :].rearrange(
                                "r p s -> p r s"))
                        st[hp] = g
                uL.append(loadg)
                units.append(uL)
                for oc2 in range(2):
                    unit = deque()
                    for idx in range(KCH):
                        r, hp = idx // 2, idx % 2
                        def mm(oc2=oc2, r=r, hp=hp, idx=idx):
                            if oc2 == 0 and idx == 0:
                                st["ps"] = psP.tile([128, 2, SQG], F32,
                                                    tag="pj", name=f"po{jg}")
                                st["osb"] = opool.tile([128, 2, SQG], F16,
                                                       tag="osb",
                                                       name=f"osb{jg}")
                            nc.tensor.matmul(
                                st["ps"][:, oc2, :],
                                lhsT=wo_sb[:, 2 * r + hp,
                                           128 * oc2:128 * (oc2 + 1)],
                                rhs=st[hp][:, r, :],
                                start=(idx == 0), stop=(idx == KCH - 1))
                        unit.append(mm)

                    def close(oc2=oc2):
                        nc.vector.tensor_copy(
                            st["osb"][:, oc2, :], st["ps"][:, oc2, :])
                    unit.append(close)
                    units.append(unit)
                uS = deque()

                def ship():
                    for h2 in range(2):
                        nc.sync.dma_start(
                            out[h2:h2 + 1, :, SQG * jg:SQG * (jg + 1)],
                            st["osb"][:, h2, :])
                uS.append(ship)
                units.append(uS)
                return units

            # ---------------- attention ----------------
            def attn_jg(jg, drain_plan=None):
                """drain_plan: optional fn(block_index_from_1) -> #thunks to
                drain after that block. Lets the schedule position AG-gated
                outproj matmuls late enough that the gather has landed by
                the time they reach the in-order PE head."""
                nsk = 4 * jg + 4 if causal else NSK
                nblk = 0
                for hp in range(2):
                    ctx_ps = [psC.tile([65, SQG], F32, tag=f"ctx{m}",
                                       name=f"ctx{jg}_{hp}_{m}")
                              for m in range(2)]
                    ets = {}

                    def pv(i):
                        et, c0 = ets.pop(i)
                        for m in range(2):
                            hl = 2 * hp + m
                            nc.tensor.matmul(
                                ctx_ps[m][:, c0:SQG],
                                lhsT=Vp[:, i, 65 * hl:65 * hl + 65],
                                rhs=et[:, m, c0:SQG],
                                start=(i == 0), stop=(i == nsk - 1))

                    for i in range(nsk):
                        col0 = 128 * max(0, i - 4 * jg) if causal else 0
                        sps = psS.tile([128, 2, SQG], F32, tag="sps",
                                       name=f"sps{jg}_{hp}_{i}")
                        for m in range(2):
                            nc.tensor.matmul(
                                sps[:, m, col0:SQG],
                                lhsT=kT[hp][64 * m:64 * m + 64,
                                            128 * i:128 * (i + 1)],
                                rhs=qT[hp][64 * m:64 * m + 64,
                                           SQG * jg + col0:SQG * (jg + 1)],
                                start=True, stop=True)
                        et = epool.tile([128, 2, SQG], F16, tag="exp",
                                        name=f"exp{jg}_{hp}_{i}")
                        nc.scalar.activation(
                            et[:, :, col0:SQG], sps[:, :, col0:SQG],
                            AF.Exp, scale=SCALE)
                        if causal and i >= 4 * jg:
                            # zero strictly-upper triangle of the diagonal
                            # 128x128 sub-block via the 0/1 tril mask. On
                            # DVE, NOT gpsimd: collective triggers block the
                            # gpsimd queue and would stall these (and with
                            # them the PV chain).
                            nc.vector.tensor_tensor(
                                et[:, :, col0:col0 + 128],
                                et[:, :, col0:col0 + 128],
                                mask01[:], ALU.mult)
                        ets[i] = (et, col0)
                        nblk += 1
                        drain(drain_plan(nblk) if drain_plan else 2)
                        if i > 1:
                            pv(i - 2)
                    if nsk > 1:
                        pv(nsk - 2)
                    pv(nsk - 1)

                    # ---- softmax finalize (head) ----
                    # Copy denominators AND raw ctx out of PSUM immediately:
                    # the ctx bank ring (bufs=1) gates the next (hp,jg)'s PV
                    # chain, so its last reader must come as early as
                    # possible. Then the reciprocal runs via an SBUF bounce
                    # (DVE recip is 8 cyc/elem -- spread the 1024
                    # denominators over 16 partitions, 64/lane). All under
                    # high_priority so the scheduler doesn't bury the hops
                    # deep in the next group's streams. The tiny DMAs ride
                    # the scalar queue: sync now carries the AG-gated ctxg
                    # loads and would strand them behind a pending gather.
                    with tc.high_priority():
                        den = opool.tile([1, 2 * SQG], F32, tag="den",
                                         name=f"den{jg}_{hp}")
                        craw = opool.tile([128, SQG], F32, tag="craw",
                                          name=f"craw{jg}_{hp}")
                        for m in range(2):
                            nc.vector.tensor_copy(
                                den[:, SQG * m:SQG * (m + 1)],
                                ctx_ps[m][64:65, :])
                            nc.vector.tensor_copy(
                                craw[64 * m:64 * m + 64, :], ctx_ps[m][0:64, :])
                        den_sp = opool.tile([16, 64], F32, tag="densp",
                                            name=f"densp{jg}_{hp}")
                        nc.scalar.dma_start(den_sp[:], den[:])
                        rec_sp = opool.tile([16, 64], F32R, tag="recsp",
                                            name=f"recsp{jg}_{hp}")
                        with nc.allow_low_precision(reason="recip in f32r"):
                            nc.vector.reciprocal(rec_sp[:], den_sp[:])
                        rec = opool.tile([1, 2 * SQG], F32R, tag="rec",
                                         name=f"rec{jg}_{hp}")
                        nc.scalar.dma_start(rec[:], rec_sp[:])
                    # ---- finalize tail: DEFERRED as a filler unit ----
                    # The bc broadcast matmuls would otherwise sit in the
                    # in-order PE stream right at the hp boundary, stalling
                    # the PE ~4-6us while the den->recip->rec DMA chain
                    # lands (measured at every finalize). As fillers they
                    # drain a few blocks into the next attention stretch,
                    # by when rec is long ready. bc comes from the psS
                    # (scores) ring, NOT psP: the psP ring chained every
                    # finalize behind the in-flight outproj unit and
                    # serialized the whole tail.
                    def fin_tail(jg=jg, hp=hp, craw=craw, rec=rec):
                        bc = psS.tile([128, 2, SQG], F32, tag="sps",
                                      name=f"bc{jg}_{hp}")
                        for m in range(2):
                            nc.tensor.matmul(bc[0:64, m, :],
                                             lhsT=ones64[:],
                                             rhs=rec[:, SQG * m:SQG * (m + 1)],
                                             start=True, stop=True)
                            nc.vector.tensor_tensor(
                                ctx_sb[64 * m:64 * m + 64, hp,
                                       SQG * jg:SQG * (jg + 1)],
                                craw[64 * m:64 * m + 64, :],
                                bc[0:64, m, :], ALU.mult)
                    if hp == 0:
                        fillers.append(deque([fin_tail]))
                    else:
                        pending_tail.append(
                            deque([fin_tail, lambda jg=jg: ship_ctx(jg)]))
                    drain_unit_boundary()

            # ---------------- schedule ----------------
            # outproj(jg) is deferred TWO attention groups (fillers of
            # attn(jg+2)): the cores launch with tens of us of skew and
            # each AllGather completes only when the LAST rank has shipped,
            # so a gather's consumers need a full attention group of slack
            # or they stall the in-order PE stream (measured 16-24us gaps).
            # Only the k/q units of proj(0) run as prologue; proj(0)'s v
            # units AND all of proj(1) drain inside attn(0) at 5/block so
            # the first exp fires right after the q projection and the
            # attn(0)->attn(1) boundary has no serial proj bubble. (The v
            # units must fully drain by attn(0) block 4: pv(nsk-2) of hp=0
            # consumes Vp entries produced by the last v thunk.)
            p0 = proj_units(0)
            for u in p0[:2]:
                while u:
                    u.popleft()()
            fillers.extend(p0[2:])
            for jg in range(NSQG):
                # everything queued before this point produces data attn(jg)
                # may read (proj of group jg) -- it must precede attn(jg) in
                # each engine's in-order stream or the PE queue deadlocks.
                if jg > 0:
                    drain_all()
                while pending_tail:
                    fillers.append(pending_tail.popleft())
                plan = None
                if jg == 0:
                    plan = lambda b: 5
                if jg + 1 < NSQG:
                    fillers.extend(proj_units(jg + 1))
                if jg == 1:
                    # hold the first drains so fin-tail(0,1)'s bc matmuls
                    # reach the PE head only after its recip chain landed
                    plan = lambda b: 0 if b <= 2 else (5 if b == 3 else 2)
                if jg == 2:
                    op_units = outproj_units(0)
                    fillers.append(op_units[0])   # loadg: sync-queue only
                    fillers.extend(op_units[1:])
                    # fin-tail(1,1) at block 3; proj(3) blocks 4-20;
                    # outproj(0) blocks ~20-30, by when AG(0) (fired ~1.5
                    # groups earlier) has landed.
                    plan = lambda b: 0 if b <= 2 else (5 if b == 3 else 2)
                if jg == 3:
                    op1 = outproj_units(1)
                    op2 = outproj_units(2)
                    fillers.append(op1[0])
                    fillers.extend(op1[1:])
                    fillers.append(op2[0])
                    fillers.extend(op2[1:])
                    # fin-tail(2,1)+loadg(1) at block 3; AG(1) lands early
                    # in attn(3), AG(2) ~2/3 through: defer outproj(1) to
                    # blocks 23+, outproj(2) to ~33+.
                    plan = lambda b: 0 if b <= 2 else (
                        3 if b == 3 else (0 if b < 23 else 2))
                attn_jg(jg, plan)
            drain_all()
            # PE warm-keepers: real-shaped but unread matmuls that execute
            # during the fin-tail(3,1) recip chain and the AG(3) wait.
            # Without them the PE sits idle >3.4us, the HAM re-throttles to
            # 1.2GHz, and the critical-tail matmuls run at half rate.
            warm = psS.tile([128, 2, SQG], F32, tag="sps", name="warmup")
            for w in range(10):
                nc.tensor.matmul(
                    warm[:, w % 2, :], lhsT=wo_sb[:, 0, 0:128],
                    rhs=ctx_sb[:, 0, 0:SQG], start=True, stop=True)
            while pending_tail:            # fin-tail(3,1) + ship/AG(3)
                u = pending_tail.popleft()
                while u:
                    u.popleft()()
            warm2 = psS.tile([128, 2, SQG], F32, tag="sps", name="warmup2")
            for w in range(14):
                nc.tensor.matmul(
                    warm2[:, w % 2, :], lhsT=wo_sb[:, 0, 0:128],
                    rhs=ctx_sb[:, 0, 0:SQG], start=True, stop=True)
            for u in outproj_units(NSQG - 1):
                while u:
                    u.popleft()()

            if debug:
                for hp in range(2):
                    nc.sync.dma_start(d_qT[hp], qT[hp][:])
                    nc.sync.dma_start(d_kT[hp], kT[hp][:])
                    nc.sync.dma_start(d_ctx[:, hp, :], ctx_sb[:, hp, :])
                nc.sync.dma_start(d_Vp[:], Vp[:])

    _split_multiwait(nc)
    return nc


def _mask_kind(mask: np.ndarray) -> bool:
    """True if causal (tril), False if all-ones; raises otherwise."""
    m = np.asarray(mask).reshape(S, S)
    if np.array_equal((m != 0).astype(np.int8), np.tril(np.ones((S, S), np.int8))):
        return True
    if np.all(m != 0):
        return False
    raise NotImplementedError("unsupported mask pattern")


def kernel(q, k, v, mask, w_q, b_q, w_k, b_k, w_v, b_v, w_o, b_o):
    global LAST_RESULT
    assert not np.any(b_q) and not np.any(b_k) and not np.any(b_v) \
        and not np.any(b_o), "nonzero biases not supported"
    causal = _mask_kind(mask)

    if causal not in _CACHE:
        _CACHE[causal] = _build(causal)
    nc = _CACHE[causal]

    f8 = np.float16

    def pack_x(xt):
        # [S, D] -> [128, NSQG, KCH, SQG]: contiguous 8KB per (partition,
        # group) so each staging DMA runs at full HBM line rate.
        t = np.asarray(xt, np.float32).T * XS          # [D, S]
        t = t.reshape(KCH, 128, NSQG, SQG).transpose(1, 2, 0, 3)
        return np.ascontiguousarray(t).astype(f8)

    def pack_w(wf, g):
        # [D, DL-slice] -> [128, KCH, DL] contiguous.
        t = wf[:, DL * g:DL * (g + 1)].reshape(KCH, 128, DL).transpose(1, 0, 2)
        return np.ascontiguousarray(t)

    xqs = [pack_x(q[b]) for b in range(B)]
    xks = [pack_x(k[b]) for b in range(B)]
    xvs = [pack_x(v[b]) for b in range(B)]
    # q/k/v weights scaled by WS; w_o folds away the v-path's XS*WS
    wqf = np.asarray(w_q, np.float32) * WS
    wkf = np.asarray(w_k, np.float32) * WS
    wvf = np.asarray(w_v, np.float32) * WS
    wof = np.asarray(w_o, np.float32) / PSC
    wqs = [pack_w(wqf, g).astype(f8) for g in range(4)]
    wks = [pack_w(wkf, g).astype(f8) for g in range(4)]
    wvs = [pack_w(wvf, g).astype(f8) for g in range(4)]
    wos = [pack_w(wof, g).astype(np.float16) for g in range(4)]

    in_maps = []
    for c in range(NCORE):
        b, g = c // 4, c % 4
        in_maps.append({
            "xq": xqs[b], "xk": xks[b], "xv": xvs[b],
            "wq": wqs[g], "wk": wks[g], "wv": wvs[g], "wo": wos[g],
        })
    res = run_bass_kernel_spmd(nc, in_maps, core_ids=list(range(NCORE)))
    LAST_RESULT = res

    outf = np.empty((B, S, D), np.float32)
    for c in range(NCORE):
        b, g = c // 4, c % 4
        o = res.results[c]["out"].reshape(DL, S).astype(np.float32)
        outf[b, :, DL * g:DL * (g + 1)] = o.T
    return outf

